# revision 31
# baseline (speedup 1.0000x reference)
"""Trainium2 Bass kernel for MockFP8Linear: out = x @ (W * block_scale)^T.

Strategy: data-parallel over tokens across 8 NeuronCores (no collectives).

Layout: the PE contracts along the partition dim, so both operands need
in_features on partitions. Both are fed to the device pre-transposed as
host-side layout prep (np.ascontiguousarray(.T) + bf16 cast, exactly the
prep class the baseline already used for W):
  - weight: [in, out] bf16. Dequant (per-128x128-block scale) happens
    on-device in one DVE tensor_tensor multiply per k-row, using a
    stride-0 broadcast AP for the scales. W^T (bf16, 8 MB) stays
    resident in SBUF.
  - x: tile-blocked transposed bf16 per-core shard, xb[t, p, kb, m] =
    x[t*128+m, kb*128+p], so each token tile is ONE [128, 4KB-run] DMA
    (DMA engines are packet-rate bound: 256B-run block DMAs measured
    ~6 GB/s/engine vs ~24 GB/s at 4KB runs) and lhsT blocks slice
    straight out of SBUF. No on-device transpose or cast: the
    TensorEngine runs a pure matmul stream.

Main compute runs as TWO PASSES over output halves so the prologue only
gates on half the W dequant: pass A computes out[:, 0:1024] for all 16
token tiles (the first four tiles interleaved k-block-by-k-block so the
PE chases the W-half-row DMA arrivals), pass B computes out[:, 1024:]
as a pure matmul stream over the fully resident operands. Per (tile,
k-block): lhsT(=x^T block, stationary) @ rhs(=W^T slice, moving, N=512)
bf16 matmuls accumulate fp32 into 2 PSUM banks per tile-half (4 tile
groups in flight). The W second-half DMA triggers and dequants are
woven into pass A's eviction stream, far off the critical path. All
dequant runs on DVE (GPSIMD tensor_tensor measured 2.5x-slowing
concurrent DVE ops; ACT's scale operand can't vary along free dims).
DVE/ACT split the PSUM eviction per chunk; each tile-half is DMA'd out
from SBUF staging via one gpsimd-issued trigger, and the last tile-half
is chunk-pipelined to shrink the drain tail.
"""

import os
import sys

import numpy as np

for _p in ("/opt/trn_rl_repo", "/root/.axon_site/_ro/trn_rl_repo"):
    if os.path.isdir(_p) and _p not in sys.path:
        sys.path.append(_p)

TOKENS, IN_F, OUT_F = 16384, 2048, 2048
NCORES = 8
TSH = TOKENS // NCORES  # tokens per core
P = 128
KB = IN_F // P  # contraction blocks
TB = TSH // P  # token tiles per core
OBL = OUT_F // P  # out_features blocks (scale granularity)
NCH = OUT_F // 512  # psum chunks of the output row-tile

_cached = None


def _build():
    from contextlib import ExitStack

    import concourse.tile as tile
    from concourse import bacc, mybir
    from concourse.bass import ds

    f32 = mybir.dt.float32
    bf16 = mybir.dt.bfloat16

    nc = bacc.Bacc("TRN2", target_bir_lowering=False, debug=False, num_devices=NCORES)
    xb_d = nc.dram_tensor("xb", [TB * P, IN_F], bf16, kind="ExternalInput").ap()
    wt_d = nc.dram_tensor("wt", [IN_F, OUT_F], bf16, kind="ExternalInput").ap()
    s_d = nc.dram_tensor("s", [P, KB, OBL], f32, kind="ExternalInput").ap()
    o_d = nc.dram_tensor("out", [TSH, OUT_F], f32, kind="ExternalOutput").ap()

    with tile.TileContext(nc) as tc:
        with ExitStack() as ctx:
            const = ctx.enter_context(tc.tile_pool(name="const", bufs=1))
            scales = const.tile([P, KB, OBL], f32)
            nc.scalar.dma_start(scales[:], s_d[:])

            wT_pool = ctx.enter_context(tc.tile_pool(name="wT", bufs=1))
            wTs = [wT_pool.tile([P, OUT_F], bf16, name=f"wT_{ib}") for ib in range(KB)]

            wnat_pool = ctx.enter_context(tc.tile_pool(name="wnat", bufs=6))
            x_pool = ctx.enter_context(tc.tile_pool(name="x", bufs=1))
            outsb_pool = ctx.enter_context(tc.tile_pool(name="outsb", bufs=3))
            ps_pool = ctx.enter_context(tc.tile_pool(name="ps", bufs=8, space="PSUM"))

            HW = OUT_F // 2  # output columns per pass

            def dequant(kb, src, lo, w):
                # wTs[kb][:, lo:lo+w] = src[:, 0:w] * scale; src 2D contiguous
                nb = w // P
                nc.vector.tensor_tensor(
                    out=wTs[kb][:, ds(lo, w)].rearrange("p (b c) -> p b c", c=P),
                    in0=src[:, ds(0, w)].rearrange("p (b c) -> p b c", c=P),
                    in1=scales[:, kb, ds(lo // P, nb), None].broadcast_to([P, nb, P]),
                    op=mybir.AluOpType.mult,
                )

            def emit_wa_group(kb0, nrows, trig, chunks=1):
                # one trigger (~0.7us of engine time) loads nrows rows' pass-A
                # halves; each row stays a 2D contiguous [P, HW] slice
                wg = wnat_pool.tile(
                    [P, nrows * HW], bf16, tag="wnat", name=f"wa_{kb0}"
                )
                if nrows == 1:
                    cw = HW // chunks
                    for j in range(chunks):
                        trig.dma_start(
                            wg[:, ds(j * cw, cw)],
                            wt_d[ds(kb0 * P, P), ds(j * cw, cw)],
                        )
                        dequant(kb0, wg[:, ds(j * cw, cw)], j * cw, cw)
                    return
                trig.dma_start(
                    wg[:].rearrange("p (r o) -> p r o", o=HW),
                    wt_d[ds(kb0 * P, nrows * P), ds(0, HW)].rearrange(
                        "(r p) o -> p r o", p=P
                    ),
                )
                for r in range(nrows):
                    dequant(kb0 + r, wg[:, ds(r * HW, HW)], 0, HW)

            wbs = {}

            def emit_wb_load(kb0, nrows):
                # pass-B halves: queued on gpsimd right AFTER the critical
                # pass-A payload (ordered queues => no early contention)
                wg = wnat_pool.tile(
                    [P, nrows * HW], bf16, tag="wnat", name=f"wb_{kb0}"
                )
                nc.gpsimd.dma_start(
                    wg[:].rearrange("p (r o) -> p r o", o=HW),
                    wt_d[ds(kb0 * P, nrows * P), ds(HW, HW)].rearrange(
                        "(r p) o -> p r o", p=P
                    ),
                )
                for r in range(nrows):
                    wbs[kb0 + r] = wg[:, ds(r * HW, HW)]

            def dequant_b_row(kb):
                # pass-B dequant on ACT as per-128-block muls with a [P,1]
                # per-partition scale AP (~0.5us each, ACT is otherwise
                # idle, and DVE must stay evictions-only)
                src_row = wbs[kb]
                for bo in range(OBL // 2):
                    nc.scalar.mul(
                        wTs[kb][:, ds(HW + bo * P, P)],
                        src_row[:, ds(bo * P, P)],
                        scales[:, kb, HW // P + bo, None],
                    )

            xtiles = {}

            def emit_x_tile(t, trig=None):
                xt = x_pool.tile([P, IN_F], bf16, name=f"x_{t}")
                (trig or nc.sync).dma_start(xt[:], xb_d[ds(t * P, P), :])
                xtiles[t] = xt

            psums = {}

            def open_group(t):
                psums[t] = [
                    ps_pool.tile([P, 512], f32, tag="ps", name=f"ps_{t}_{c}")
                    for c in range(2)
                ]

            def mm_one(t, kb, half, c):
                nc.tensor.matmul(
                    psums[t][c][:],
                    lhsT=xtiles[t][:, ds(kb * P, P)],
                    rhs=wTs[kb][:, ds(half * HW + c * 512, 512)],
                    start=(kb == 0),
                    stop=(kb == KB - 1),
                )

            def mm(t, kb, half):
                mm_one(t, kb, half, 0)
                mm_one(t, kb, half, 1)

            def close_tile(t, half):
                # both evictions on DVE: start=True matmuls wait on coarsened
                # DVE op-COUNT semaphores, so the eviction engine's stream
                # must contain nothing slower-paced than evictions
                outsb = outsb_pool.tile([P, HW], f32, tag="osb", name=f"osb_{t}_{half}")
                nc.vector.tensor_copy(outsb[:, ds(0, 512)], psums[t][0][:])
                nc.vector.tensor_copy(outsb[:, ds(512, 512)], psums[t][1][:])
                # all outs ride the sync queue: demand-paced (~0.5MB per
                # 6.8us), and keeping them off gpsimd avoids head-of-line
                # blocking behind the pass-B W quads
                nc.sync.dma_start(o_d[ds(t * P, P), ds(half * HW, HW)], outsb[:])
                del psums[t]

            # ---- prologue: the critical payload (x0-x5 + pass-A W halves,
            # ~7MB) is spread across all three DMA queues — each queue tops
            # out at ~180 GB/s, so one queue alone would gate the prologue
            # at ~30us. Row 0 is chunked for the earliest first matmul;
            # grouped W DMAs keep trigger serialization (~0.7us each) off
            # the pacing path.
            # Each queue moves only ~100 GB/s when all three are active, so
            # the payload is striped round-robin across sync/scalar/gpsimd
            # in strict consumption order (the DVE wait conditions are
            # coarsened to op-counts, so out-of-order arrival cascades).
            # x0/x1 are split into k-strips so their first k-blocks land in
            # ~1us instead of ~5us.
            # the gpsimd queue delivers its first bytes ~10us later than
            # sync/scalar, so the critical payload rides those two only
            def emit_x_striped(t):
                xt = x_pool.tile([P, IN_F], bf16, name=f"x_{t}")
                nc.sync.dma_start(xt[:, ds(0, 1024)], xb_d[ds(t * P, P), ds(0, 1024)])
                nc.scalar.dma_start(
                    xt[:, ds(1024, 1024)], xb_d[ds(t * P, P), ds(1024, 1024)]
                )
                xtiles[t] = xt

            emit_x_striped(0)
            emit_wa_group(0, 1, nc.sync)
            emit_wa_group(1, 1, nc.scalar)
            emit_x_striped(1)
            for kb in range(2, KB):
                emit_wa_group(kb, 1, nc.sync if kb % 2 == 0 else nc.scalar)
            emit_x_tile(2)                      # sync
            emit_x_tile(3, nc.scalar)
            emit_x_tile(4, nc.gpsimd)
            emit_x_tile(5)                      # sync

            # PE/HAM warmup: a few fp32 matmuls off the scales tile (first
            # data to arrive) into a scratch PSUM bank, so the HAM clock
            # gate is already at 8/8 when the real stream starts
            sflat = scales[:].rearrange("p a b -> p (a b)")
            warm = ps_pool.tile([P, 512], f32, tag="ps", name="warm")
            for _ in range(5):
                nc.tensor.matmul(
                    warm[:, ds(0, 256)], lhsT=sflat[:, ds(0, P)],
                    rhs=sflat[:, ds(0, 256)], start=True, stop=True,
                )

            # ---- pass A over out[:, 0:1024]: first four tiles interleaved
            # k-block-by-k-block (8 matmuls = ~1.7us of PE work per arriving
            # W row) so the PE never starves during the W load phase.
            for t in range(4):
                open_group(t)
            for kb in range(KB):
                for t in range(4):
                    mm(t, kb, 0)

            def weave(t):
                # spread the x-tile bulk and pass-B W quads across pass A's
                # eviction stream, paced so each quad is in before its ACT
                # dequant rows come up and each x tile before its matmuls:
                # gpsimd carries x6-x9 + the quads, sync carries x10-x15
                # (ahead of that close's demand-paced out-DMA)
                if t == 0:
                    emit_x_tile(6, nc.gpsimd)
                    emit_x_tile(7, nc.gpsimd)
                    emit_wb_load(0, 4)
                elif t == 1:
                    emit_wb_load(4, 4)
                    emit_x_tile(8, nc.gpsimd)
                elif t == 2:
                    emit_wb_load(8, 4)
                    emit_x_tile(9, nc.gpsimd)
                    emit_x_tile(10)
                    emit_x_tile(11)
                elif t == 3:
                    emit_wb_load(12, 4)
                    emit_x_tile(12)
                    emit_x_tile(13)
                elif t == 4:
                    emit_x_tile(14)
                    emit_x_tile(15)
                if 2 * t + 1 < KB:
                    dequant_b_row(2 * t)
                    dequant_b_row(2 * t + 1)

            for t in range(4):
                weave(t)
                close_tile(t, 0)

            for t in range(4, TB):
                open_group(t)
                for kb in range(KB):
                    mm(t, kb, 0)
                weave(t)
                close_tile(t, 0)

            # ---- pass B over out[:, 1024:2048]: pure matmul stream ----
            for t in range(TB):
                open_group(t)
                last = t == TB - 1
                if not last:
                    for kb in range(KB):
                        mm(t, kb, 1)
                    close_tile(t, 1)
                else:
                    # chunk-outer on the final tile so the drain pipelines
                    outsb = outsb_pool.tile([P, HW], f32, tag="osb", name="osb_last")
                    for c in range(2):
                        for kb in range(KB):
                            mm_one(t, kb, 1, c)
                        if c == 0:
                            nc.vector.tensor_copy(
                                outsb[:, ds(0, 512)], psums[t][0][:]
                            )
                            nc.gpsimd.dma_start(
                                o_d[ds(t * P, P), ds(HW, 512)], outsb[:, ds(0, 512)]
                            )
                        else:
                            # final chunk: evict + drain split across queues
                            nc.vector.tensor_copy(
                                outsb[:, ds(512, 256)], psums[t][1][:, ds(0, 256)]
                            )
                            nc.scalar.copy(
                                outsb[:, ds(768, 256)], psums[t][1][:, ds(256, 256)]
                            )
                            nc.sync.dma_start(
                                o_d[ds(t * P, P), ds(HW + 512, 256)],
                                outsb[:, ds(512, 256)],
                            )
                            nc.scalar.dma_start(
                                o_d[ds(t * P, P), ds(HW + 768, 256)],
                                outsb[:, ds(768, 256)],
                            )
                    del psums[t]

    nc.compile()
    return nc


def _get_compiled():
    global _cached
    if _cached is None:
        _cached = _build()
    return _cached


def _ensure_ntff_hook():
    """Register the axon NTFF profile hook (boot skips it when
    antenv.axon_hooks is absent from the image). Only needed for trace=True."""
    import sys as _sys
    import types as _types

    if "antenv.axon_hooks" not in _sys.modules:
        import antenv

        mod = _types.ModuleType("antenv.axon_hooks")
        mod._hook = None

        def set_axon_ntff_profile_hook(h):
            mod._hook = h

        def get_axon_ntff_profile_hook():
            return mod._hook

        mod.set_axon_ntff_profile_hook = set_axon_ntff_profile_hook
        mod.get_axon_ntff_profile_hook = get_axon_ntff_profile_hook
        _sys.modules["antenv.axon_hooks"] = mod
        antenv.axon_hooks = mod
    mod = _sys.modules["antenv.axon_hooks"]
    if mod._hook is None:
        from trn_agent_boot.trn_boot import _ntff_profile_via_ctypes

        hook = _ntff_profile_via_ctypes("/opt/axon/libaxon_pjrt.so")
        if hook is not None:
            mod.set_axon_ntff_profile_hook(hook)


def run(x, weight, weight_scale, trace=False, trace_cores=None):
    from concourse.bass_utils import run_bass_kernel_spmd

    nc = _get_compiled()

    import ml_dtypes

    bf16 = ml_dtypes.bfloat16
    x = np.asarray(x, dtype=np.float32)
    weight = np.asarray(weight, dtype=np.float32)
    wt = np.ascontiguousarray(weight.T.astype(bf16))
    weight_scale = np.asarray(weight_scale, dtype=np.float32)
    # [P, KB(bi), OBL(bo)]: s[p, bi, bo] = weight_scale[bo, bi]
    scales_b = np.ascontiguousarray(
        np.broadcast_to(weight_scale.T[None, :, :], (P, KB, OBL)).astype(np.float32)
    )

    def blocked_x(shard):
        # xb[t, p, kb, m] = shard[t*128+m, kb*128+p]  (layout prep only)
        xb = shard.reshape(TB, P, KB, P).transpose(0, 3, 2, 1)
        return np.ascontiguousarray(xb.astype(bf16).reshape(TB * P, IN_F))

    in_maps = [
        {
            "xb": blocked_x(x[c * TSH : (c + 1) * TSH]),
            "wt": wt,
            "s": scales_b,
        }
        for c in range(NCORES)
    ]
    kwargs = {}
    if trace:
        try:
            _ensure_ntff_hook()
        except Exception as e:  # tracing is best-effort; the run still works
            print(f"ntff hook registration failed ({e}); tracing may be skipped")
        kwargs = dict(trace=True, trace_cores=trace_cores or [0])
    res = run_bass_kernel_spmd(nc, in_maps, core_ids=list(range(NCORES)), **kwargs)
    out = np.concatenate([res.results[c]["out"] for c in range(NCORES)], axis=0)
    return out, res


def kernel(x, weight, weight_scale):
    # Rare transient device errors (NRT_EXEC_UNIT_UNRECOVERABLE) have been
    # observed under the profiling path; retry once to be safe.
    try:
        out, _ = run(x, weight, weight_scale)
    except Exception:
        import time

        time.sleep(2)
        out, _ = run(x, weight, weight_scale)
    return out
